# revision 36
# baseline (speedup 1.0000x reference)
"""Multi-head causal self-attention (B=2, T=2048, D=2048, 16 heads, RoPE)
on 8 Trainium2 NeuronCores.

Sharding strategy
-----------------
* Phase 1+2 (QKV projection, RoPE, attention): tensor-parallel over heads —
  each core owns 2 of the 16 heads for both batch elements. Each core reads
  the full (host-transposed, bf16) x and only its slice of qkv_w, and
  computes q/k in a transposed [head_dim, t] layout so RoPE and the score
  matmuls need no on-device transposes.
* Attention is computed as sT[tk, tq] = k-tile^T-free @ q (so softmax sums
  are ones-matmuls on the PE and the attention output lands directly in
  oT[dv, t] layout), exp on the scalar engine, probabilities in bf16.
  All matmul operands are bf16 (weights, activations, probabilities);
  PSUM accumulation stays fp32, softmax denominators stay fp32.
* Phase 3: four on-device AllToAlls — one per (local head, batch) — swap
  the head-sharded oT into a sequence-sharded full-head oT (0.5 MiB/core
  each in bf16). Each fires as soon as its (head, batch) attention
  completes, so only the very last one is tail-exposed.
* Phase 4: output projection is data-parallel over rows — core c owns 256
  rows of each batch. It runs per batch: batch-0's projection overlaps
  batch-1's QKV/attention compute. out_w is resident in SBUF in bf16 and
  loaded once. Output bias is folded into the even-parity parking add on
  the vector engine. Host concatenates the 8 row-slices per batch.
"""

import numpy as np

B = 2
T = 2048
D = 2048
H = 16            # global heads
HD = 128          # head dim
NCORES = 8
HPC = H // NCORES  # heads per core
NKT = D // 128     # contraction tiles over the embedding dim
TCH = 512          # t-chunk (phase 1) / tq-chunk (phase 2) width
SCALE = 1.0 / np.sqrt(HD)

_CACHE = {}


def _build_module(t_total=T):
    import concourse.bacc as bacc
    import concourse.mybir as mybir
    import concourse.tile as tile

    F32 = mybir.dt.float32
    F32R = mybir.dt.float32r
    BF16 = mybir.dt.bfloat16
    ADD = mybir.AluOpType.add
    MULT = mybir.AluOpType.mult
    AF = mybir.ActivationFunctionType

    t_ch = t_total // TCH          # chunks per batch element (4)
    rows = B * t_total             # 4096
    rpc = rows // NCORES           # rows per core (512)
    rpb = rpc // B                 # rows per core per batch (256)
    n_rt = rpb // 128              # row tiles per core per batch (2)
    w = HPC * HD                   # per-core q/k/v feature width (256)
    n_fc = D // TCH                # output-projection column chunks (4)

    nc = bacc.Bacc("TRN2", target_bir_lowering=False, debug=False,
                   num_devices=NCORES)

    # ---- I/O ----
    xT = nc.dram_tensor("xT", [B, D, t_total], BF16, kind="ExternalInput")
    wq = nc.dram_tensor("wq", [D, w], BF16, kind="ExternalInput")
    wk = nc.dram_tensor("wk", [D, w], BF16, kind="ExternalInput")
    wv = nc.dram_tensor("wv", [D, w], BF16, kind="ExternalInput")
    bqk = nc.dram_tensor("bqk", [HD, 2 * HPC], F32, kind="ExternalInput")
    bv = nc.dram_tensor("bv", [1, 2 * w], BF16, kind="ExternalInput")
    wo = nc.dram_tensor("wo", [D, D], BF16, kind="ExternalInput")
    bo = nc.dram_tensor("bo", [128, D], BF16, kind="ExternalInput")
    cosT = nc.dram_tensor("cosT", [HD, t_total], BF16, kind="ExternalInput")
    sinT = nc.dram_tensor("sinT", [HD, t_total], BF16, kind="ExternalInput")
    pt = nc.dram_tensor("pt", [HD, HD], BF16, kind="ExternalInput")
    maskT = nc.dram_tensor("maskT", [HD, HD], BF16, kind="ExternalInput")
    onec = nc.dram_tensor("onec", [HD, 1], BF16, kind="ExternalInput")
    oner = nc.dram_tensor("oner", [1, HD], BF16, kind="ExternalInput")
    y = nc.dram_tensor("y", [rpc, D], F32, kind="ExternalOutput")

    with tile.TileContext(nc) as tc:
        frees = []

        def single(shape, dtype, name, flist=frees):
            t, free = tc.tile(shape, dtype, name=name)
            flist.append(free)
            return t

        # ---- constants + resident weights in SBUF ----
        cos_sb = single([HD, t_total], BF16, "cos_sb")
        sin_sb = single([HD, t_total], BF16, "sin_sb")
        pt_sb = single([HD, HD], BF16, "pt_sb")
        mask_sb = single([HD, HD], BF16, "mask_sb")
        onec_sb = single([HD, 1], BF16, "onec_sb")
        oner_sb = single([1, HD], BF16, "oner_sb")
        bqk_sb = single([HD, 2 * HPC], F32, "bqk_sb")
        bv_sb = single([1, 2 * w], BF16, "bv_sb")
        bo_sb = single([128, D], BF16, "bo_sb")
        # out_w resident for the whole kernel: [128, kt*2048 + fc*512] bf16
        wo_sb = single([128, NKT * D], BF16, "wo_sb")

        # All constant loads are deferred to after the first x-chunk DMAs
        # (below) so the first matmuls are not starved behind weight DMA.

        # ---- DRAM bounce buffers for the per-(head, batch) AllToAlls ----
        with tc.tile_pool(name="dram", bufs=1, space="DRAM") as dram:
            bounce_in = [[dram.tile([NCORES * HD, rpb], BF16,
                                    name=f"bounce_in{h}_{b}")
                          for b in range(B)] for h in range(HPC)]
            bounce_out = [[dram.tile([NCORES * HD, rpb], BF16,
                                     name=f"bounce_out{h}_{b}")
                           for b in range(B)] for h in range(HPC)]
            # the very last (head, batch) AllToAll is tail-exposed: split it
            # into two row-half collectives so the odd projection pass can
            # start as soon as the first half lands.
            bounce_in_sp = [dram.tile([NCORES * HD, 128], BF16,
                                      name=f"bounce_in_sp{rt}")
                            for rt in range(n_rt)]
            bounce_out_sp = [dram.tile([NCORES * HD, 128], BF16,
                                       name=f"bounce_out_sp{rt}")
                             for rt in range(n_rt)]

            # PSUM pools stay open for the whole kernel: 8 banks total
            # (qk shares slots with v; st shares with rot/bcast/out-proj).
            with tc.tile_pool(name="qk_ps", bufs=2, space="PSUM") as qk_ps, \
                 tc.tile_pool(name="rot_ps", bufs=1, space="PSUM") as rot_ps, \
                 tc.tile_pool(name="st_ps", bufs=3, space="PSUM") as st_ps, \
                 tc.tile_pool(name="ot_ps", bufs=2, space="PSUM") as ot_ps:
                v_ps = qk_ps

                with tc.tile_pool(name="xt", bufs=17) as xt_pool, \
                     tc.tile_pool(name="tmp", bufs=6) as tmp_pool, \
                     tc.tile_pool(name="et", bufs=6) as et_pool, \
                     tc.tile_pool(name="nrm", bufs=3) as nrm_pool, \
                     tc.tile_pool(name="ets", bufs=2) as ets_pool, \
                     tc.tile_pool(name="oto", bufs=5) as oto_pool, \
                     tc.tile_pool(name="oc", bufs=NKT + NKT // 2) as oc_pool, \
                     tc.tile_pool(name="osp", bufs=8) as osp_pool, \
                     tc.tile_pool(name="os", bufs=3) as os_pool:

                    wfrees = []
                    wq_sb = single([128, NKT * w], BF16, "wq_sb", wfrees)
                    wk_sb = single([128, NKT * w], BF16, "wk_sb", wfrees)
                    wv_sb = single([128, NKT * w], BF16, "wv_sb", wfrees)

                    oc = {}      # (b, kt) -> [128, rpb] bf16 oT tiles
                    osp = {}     # (b, fc, rt) -> parked even partials

                    def emit_phase1_chunk(b, j):
                        """QKV projection + RoPE for chunk j of batch b into
                        q_st/k_st/v_st (freed per batch by the caller)."""
                        if True:
                            tr = slice(j * TCH, (j + 1) * TCH)
                            xt = []
                            for kt in range(NKT):
                                xtile = xt_pool.tile([128, TCH], BF16,
                                                     name=f"xt{b}{j}_{kt}",
                                                     tag="xt")
                                nc.sync.dma_start(
                                    xtile[:],
                                    xT.ap()[b, kt * 128:(kt + 1) * 128, tr])
                                xt.append(xtile)
                                if b == 0 and j == 0:
                                    # only wq rides along with the first x
                                    # chunk — everything else is ordered
                                    # strictly by first use below.
                                    nc.sync.dma_start(
                                        wq_sb[:, kt * w:(kt + 1) * w],
                                        wq.ap()[kt * 128:(kt + 1) * 128, :])
                            if b == 0 and j == 0:
                                nc.sync.dma_start(bqk_sb[:], bqk.ap()[:, :])
                                nc.sync.dma_start(pt_sb[:], pt.ap()[:, :])
                                nc.sync.dma_start(cos_sb[:], cosT.ap()[:, :])
                                nc.sync.dma_start(sin_sb[:], sinT.ap()[:, :])
                                for kt in range(NKT):
                                    nc.sync.dma_start(
                                        wk_sb[:, kt * w:(kt + 1) * w],
                                        wk.ap()[kt * 128:(kt + 1) * 128, :])
                                nc.sync.dma_start(bv_sb[:], bv.ap()[:, :])
                                for kt in range(NKT):
                                    nc.sync.dma_start(
                                        wv_sb[:, kt * w:(kt + 1) * w],
                                        wv.ap()[kt * 128:(kt + 1) * 128, :])
                                nc.sync.dma_start(mask_sb[:], maskT.ap()[:, :])
                                nc.sync.dma_start(onec_sb[:], onec.ap()[:, :])
                                nc.sync.dma_start(oner_sb[:], oner.ap()[:, :])
                                nc.sync.dma_start(bo_sb[:], bo.ap()[:, :])
                            if b == 1:
                                # out_w (8 MiB bf16): load once, spread over
                                # batch-1's chunks — needed only by the
                                # projection passes which start later still.
                                k0 = j * (NKT // t_ch)
                                for kt in range(k0, k0 + NKT // t_ch):
                                    nc.sync.dma_start(
                                        wo_sb[:, kt * D:(kt + 1) * D],
                                        wo.ap()[kt * 128:(kt + 1) * 128, :])

                            for which, w_sb, store in (
                                ("q", wq_sb, q_st), ("k", wk_sb, k_st)):
                                for h in range(HPC):
                                    ps = qk_ps.tile([128, TCH], F32,
                                                    name=f"{which}ps{b}{j}{h}",
                                                    tag="qk")
                                    for kt in range(NKT):
                                        col = kt * w + h * HD
                                        nc.tensor.matmul(
                                            ps[:],
                                            w_sb[:, col:col + HD],
                                            xt[kt][:],
                                            start=(kt == 0),
                                            stop=(kt == NKT - 1))
                                    # bias (per-partition) + round to bf16
                                    bcol = h if which == "q" else HPC + h
                                    qtmp = tmp_pool.tile(
                                        [128, TCH], BF16,
                                        name=f"{which}t{b}{j}{h}", tag="tmp")
                                    nc.scalar.activation(
                                        qtmp[:], ps[:], AF.Identity,
                                        bias=bqk_sb[:, bcol:bcol + 1],
                                        scale=1.0)
                                    # rotate-half via permutation matmul
                                    rp = rot_ps.tile([128, TCH], F32,
                                                     name=f"rp{b}{j}{h}",
                                                     tag="rot")
                                    nc.tensor.matmul(rp[:], pt_sb[:], qtmp[:],
                                                     start=True, stop=True)
                                    t1 = tmp_pool.tile([128, TCH], BF16,
                                                       name=f"t1_{b}{j}{h}",
                                                       tag="tmp")
                                    nc.vector.tensor_tensor(
                                        t1[:], qtmp[:], cos_sb[:, tr], MULT)
                                    t2 = tmp_pool.tile([128, TCH], BF16,
                                                       name=f"t2_{b}{j}{h}",
                                                       tag="tmp")
                                    nc.vector.tensor_tensor(
                                        t2[:], rp[:], sin_sb[:, tr], MULT)
                                    nc.vector.tensor_tensor(
                                        store[h][:, tr], t1[:], t2[:], ADD)

                            # v in natural [t, dv] layout, two t-tiles/psum
                            for half in range(2):
                                pv = v_ps.tile([128, TCH], F32,
                                               name=f"vps{b}{j}{half}",
                                               tag="qk")
                                # bias for both column halves in one matmul
                                nc.tensor.matmul(
                                    pv[:], oner_sb[:], bv_sb[:],
                                    start=True, stop=False,
                                    skip_group_check=True)
                                for sub in range(2):
                                    ts = half * 2 + sub
                                    cs = sub * w
                                    for kt in range(NKT):
                                        nc.tensor.matmul(
                                            pv[:, cs:cs + w],
                                            xt[kt][:, ts * 128:(ts + 1) * 128],
                                            wv_sb[:, kt * w:(kt + 1) * w],
                                            start=False,
                                            stop=(kt == NKT - 1),
                                            skip_group_check=True)
                                for sub in range(2):
                                    ts = half * 2 + sub
                                    tt = j * 4 + ts   # global t-tile index
                                    for h in range(HPC):
                                        nc.scalar.copy(
                                            v_st[h][:, tt * 128:
                                                    (tt + 1) * 128],
                                            pv[:, sub * w + h * HD:
                                               sub * w + (h + 1) * HD])

                    def emit_attn_chunk(b, h, c, state):
                        """Chunk c of causal attention for (batch b, local
                        head h) — needs q of chunk c and k/v of chunks
                        0..c only, so it can interleave right behind the
                        phase-1 chunk that produced them. Each chunk's
                        denominator / normalize / scatter chain is deferred
                        until after the next chunk's first score matmuls so
                        the PE is not stalled on the DVE accumulate at
                        chunk boundaries. The last chunk scatters and fires
                        this (h, b) AllToAll."""
                        if True:
                            ot = ot_ps.tile([128, TCH], F32,
                                            name=f"ot{b}{h}{c}", tag="ot")
                            ets = ets_pool.tile([128, TCH], BF16,
                                                name=f"ets{b}{h}{c}",
                                                tag="ets")
                            kmax = 4 * c + 3
                            ets_of = {}

                            def emit_st(k, c=c, ets=ets, ets_of=ets_of):
                                """Score matmul + exp + mask + denominator
                                accumulate for k-block k."""
                                off = max(0, (k - 4 * c) * 128)
                                ksl = slice(k * 128, (k + 1) * 128)
                                st = st_ps.tile([128, TCH], F32,
                                                name=f"st{b}{h}{c}{k}",
                                                tag="st")
                                q0 = c * TCH
                                nc.tensor.matmul(
                                    st[:, off:TCH],
                                    k_st[h][:, ksl],
                                    q_st[h][:, q0 + off:q0 + TCH],
                                    start=True, stop=True,
                                    skip_group_check=True)
                                et = et_pool.tile([128, TCH], BF16,
                                                  name=f"et{b}{h}{c}{k}",
                                                  tag="et")
                                nc.scalar.activation(
                                    et[:, off:TCH], st[:, off:TCH],
                                    AF.Exp, bias=0.0, scale=float(SCALE))
                                if k >= 4 * c:
                                    # zero the not-yet-causal triangle
                                    nc.vector.tensor_tensor(
                                        et[:, off:off + 128],
                                        et[:, off:off + 128],
                                        mask_sb[:], MULT)
                                # denominator partial sums on the DVE
                                if k == 0:
                                    nc.vector.tensor_copy(ets[:], et[:])
                                else:
                                    nc.vector.tensor_tensor(
                                        ets[:, off:TCH],
                                        ets[:, off:TCH],
                                        et[:, off:TCH], ADD)
                                ets_of[k] = (et, off)

                            # software-pipelined: the score matmul for k+1
                            # is emitted before the AV matmul for k, so the
                            # in-order PE has ready work while exp(k) is
                            # still on the scalar engine.
                            emit_st(0)
                            if state["fin"] is not None:
                                state["fin"]()
                                state["fin"] = None
                            for k in range(kmax + 1):
                                if k < kmax:
                                    emit_st(k + 1)
                                et, off = ets_of.pop(k)
                                ksl = slice(k * 128, (k + 1) * 128)
                                nc.tensor.matmul(
                                    ot[:, off:TCH],
                                    v_st[h][:, ksl],
                                    et[:, off:TCH],
                                    start=(k == 0), stop=(k == kmax),
                                    skip_group_check=True)

                            def finalize(c=c, ot=ot, ets=ets):
                                # den shares the rot/bc bank — their
                                # lifetimes never overlap
                                den = rot_ps.tile([1, TCH], F32,
                                                  name=f"den{b}{h}{c}",
                                                  tag="rot")
                                nc.tensor.matmul(
                                    den[0:1, :], onec_sb[:], ets[:],
                                    start=True, stop=True,
                                    skip_group_check=True)
                                # normalize by the softmax denominator
                                rc = nrm_pool.tile([1, TCH], F32,
                                                   name=f"rc{b}{h}{c}",
                                                   tag="rc")
                                rscr = nrm_pool.tile([1, TCH], F32,
                                                     name=f"rscr{b}{h}{c}",
                                                     tag="rc")
                                nc.vector.reciprocal_approx_accurate(
                                    rc[:], den[0:1, :], rscr[:])
                                rcr = nrm_pool.tile([1, TCH], BF16,
                                                    name=f"rcr{b}{h}{c}",
                                                    tag="rcr")
                                nc.scalar.copy(rcr[:], rc[:])
                                bc = rot_ps.tile([128, TCH], F32,
                                                 name=f"bc{b}{h}{c}",
                                                 tag="rot")
                                nc.tensor.matmul(bc[:], oner_sb[:], rcr[:],
                                                 start=True, stop=True,
                                                 skip_group_check=True)
                                bcs = nrm_pool.tile([128, TCH], F32,
                                                    name=f"bcs{b}{h}{c}",
                                                    tag="bcs")
                                nc.scalar.copy(bcs[:], bc[:])
                                otn = oto_pool.tile([128, TCH], BF16,
                                                    name=f"otn{b}{h}{c}",
                                                    tag="otn")
                                nc.vector.tensor_tensor(
                                    otn[:], ot[:], bcs[:], MULT)
                                # chunk c covers cores 2c, 2c+1 of batch b
                                split = (b == B - 1 and h == HPC - 1)
                                for s2 in range(2):
                                    s = 2 * c + s2
                                    if split:
                                        for rt in range(n_rt):
                                            nc.sync.dma_start(
                                                bounce_in_sp[rt][
                                                    s * HD:(s + 1) * HD, :],
                                                otn[:,
                                                    s2 * rpb + rt * 128:
                                                    s2 * rpb +
                                                    (rt + 1) * 128])
                                    else:
                                        nc.sync.dma_start(
                                            bounce_in[h][b][
                                                s * HD:(s + 1) * HD, :],
                                            otn[:, s2 * rpb:(s2 + 1) * rpb])

                            if c < t_ch - 1:
                                state["fin"] = finalize
                                return
                            finalize()
                        if b == B - 1 and h == HPC - 1:
                            for rt in range(n_rt):
                                nc.gpsimd.collective_compute(
                                    "AllToAll",
                                    mybir.AluOpType.bypass,
                                    replica_groups=[list(range(NCORES))],
                                    ins=[bounce_in_sp[rt][:].opt()],
                                    outs=[bounce_out_sp[rt][:].opt()],
                                )
                        else:
                            nc.gpsimd.collective_compute(
                                "AllToAll",
                                mybir.AluOpType.bypass,
                                replica_groups=[list(range(NCORES))],
                                ins=[bounce_in[h][b][:].opt()],
                                outs=[bounce_out[h][b][:].opt()],
                            )

                    def emit_oc_loads(b, parities=(0, 1)):
                        """SBUF loads of the AllToAll'd oT tiles for batch b.
                        Global odim tile kt = 2*s + h (head-major)."""
                        for h in parities:
                            split = (b == B - 1 and h == HPC - 1)
                            if split:
                                # rt-major: all first-half tiles are queued
                                # before any second-half tile, so they only
                                # wait on the first half-AllToAll.
                                for rt in range(n_rt):
                                    for s in range(NCORES):
                                        kt = HPC * s + h
                                        t_ = oc_pool.tile(
                                            [128, 128], BF16,
                                            name=f"oc{b}_{kt}_{rt}",
                                            tag="oc")
                                        nc.sync.dma_start(
                                            t_[:],
                                            bounce_out_sp[rt][
                                                s * 128:(s + 1) * 128, :])
                                        oc[b, kt, rt] = t_
                                continue
                            for s in range(NCORES):
                                kt = HPC * s + h
                                t_ = oc_pool.tile([128, rpb], BF16,
                                                  name=f"oc{b}_{kt}",
                                                  tag="oc")
                                nc.sync.dma_start(
                                    t_[:],
                                    bounce_out[h][b][s * 128:(s + 1) * 128,
                                                     :])
                                oc[b, kt] = t_

                    def emit_proj_even_piece(b, fc, rt):
                        """One even-kt partial-sum tile for batch b (gated on
                        AllToAll (h=0, b)); bias folded into the parking add
                        so the psum group is pure matmul."""
                        po = st_ps.tile([128, TCH], F32,
                                        name=f"po{b}{fc}{rt}",
                                        tag="st")
                        for i in range(NCORES):
                            kt = HPC * i
                            nc.tensor.matmul(
                                po[:],
                                oc[b, kt][:, rt * 128:(rt + 1) * 128],
                                wo_sb[:, kt * D + fc * TCH:
                                      kt * D + (fc + 1) * TCH],
                                start=(i == 0),
                                stop=(i == NCORES - 1),
                                skip_group_check=True)
                        p_ = osp_pool.tile([128, TCH], BF16,
                                           name=f"osp{b}{fc}{rt}",
                                           tag="osp")
                        nc.vector.tensor_tensor(
                            p_[:], po[:],
                            bo_sb[:, fc * TCH:(fc + 1) * TCH], ADD)
                        osp[b, fc, rt] = p_

                    def emit_proj_even(b):
                        for fc in range(n_fc):
                            for rt in range(n_rt):
                                emit_proj_even_piece(b, fc, rt)

                    def emit_proj_odd_piece(b, fc, rt):
                        """One odd-kt partial-sum tile for batch b (gated on
                        AllToAll (h=1, b)), combined with the parked evens
                        on the DVE and stored to y."""
                        po = st_ps.tile([128, TCH], F32,
                                        name=f"po2_{b}{fc}{rt}",
                                        tag="st")
                        for i in range(NCORES):
                            kt = HPC * i + 1
                            if (b, kt, rt) in oc:
                                sta = oc[b, kt, rt][:, :]
                            else:
                                sta = oc[b, kt][:, rt * 128:(rt + 1) * 128]
                            nc.tensor.matmul(
                                po[:],
                                sta,
                                wo_sb[:, kt * D + fc * TCH:
                                      kt * D + (fc + 1) * TCH],
                                start=(i == 0),
                                stop=(i == NCORES - 1),
                                skip_group_check=True)
                        os_t = os_pool.tile([128, TCH], F32,
                                            name=f"os{b}{fc}{rt}",
                                            tag="osp")
                        nc.vector.tensor_tensor(
                            os_t[:], po[:], osp[b, fc, rt][:], ADD)
                        nc.sync.dma_start(
                            y.ap()[b * rpb + rt * 128:
                                   b * rpb + (rt + 1) * 128,
                                   fc * TCH:(fc + 1) * TCH],
                            os_t[:])

                    def emit_proj_odd(b):
                        """rt-outer so the first row-half of the split final
                        AllToAll unblocks work while the second lands."""
                        for rt in range(n_rt):
                            for fc in range(n_fc):
                                emit_proj_odd_piece(b, fc, rt)

                    # ================= schedule =================
                    # phase-1 chunk j and attention chunks c=j interleave:
                    # attention chunk c only needs q of chunk c and k/v of
                    # chunks 0..c. The phase-1 stream is DMA-gated and the
                    # attention stream is exp/Act-gated, so they fill each
                    # other's engine stalls. Batch-0's output projection
                    # pieces backfill batch-1's DMA-heavy phase-1 chunks.
                    proj0 = ([(emit_proj_even_piece, fr)
                              for fr in [(fc, rt) for fc in range(n_fc)
                                         for rt in range(n_rt)]] +
                             [(emit_proj_odd_piece, fr)
                              for fr in [(fc, rt) for fc in range(n_fc)
                                         for rt in range(n_rt)]])
                    # pieces per batch-1 chunk: none at j=0 (AllToAlls of
                    # batch 0 are still landing), then 5/5/6
                    proj0_sched = [proj0[0:0], proj0[0:5], proj0[5:10],
                                   proj0[10:16]]
                    for b in range(B):
                        bfrees = []
                        q_st = [single([128, t_total], BF16, f"q_st{b}{h}",
                                       bfrees) for h in range(HPC)]
                        k_st = [single([128, t_total], BF16, f"k_st{b}{h}",
                                       bfrees) for h in range(HPC)]
                        v_st = [single([128, t_total], BF16, f"v_st{b}{h}",
                                       bfrees) for h in range(HPC)]
                        states = [{"fin": None} for _ in range(HPC)]

                        for j in range(t_ch):
                            emit_phase1_chunk(b, j)
                            if b == 1 and j == 0:
                                # batch-0 oT tiles are in flight (AllToAlls
                                # fired at the end of batch 0) — bring them
                                # into SBUF behind batch-1's first x loads.
                                emit_oc_loads(0)
                            if b == 1:
                                for fn, fr in proj0_sched[j]:
                                    fn(0, *fr)
                            for h in range(HPC):
                                emit_attn_chunk(b, h, j, states[h])
                        for f in reversed(bfrees):
                            f()

                    # the even pass overlaps the split final AllToAll pair;
                    # the odd pass starts as soon as the first half lands.
                    emit_oc_loads(1, (0,))
                    emit_proj_even(1)
                    emit_oc_loads(1, (1,))
                    emit_proj_odd(1)

                    for f in reversed(wfrees):
                        f()

        for f in reversed(frees):
            f()

    nc.compile()
    return nc


def _host_inputs(x, qkv_w, qkv_b, out_w, out_b, t_total=T):
    """Build the per-core input maps (all host-side layout shuffling)."""
    import ml_dtypes

    f32 = np.float32
    bf16 = ml_dtypes.bfloat16

    x = np.asarray(x, dtype=f32)
    qkv_w = np.asarray(qkv_w, dtype=f32)
    qkv_b = np.asarray(qkv_b, dtype=f32)
    out_w = np.asarray(out_w, dtype=f32)
    out_b = np.asarray(out_b, dtype=f32)

    xT = np.ascontiguousarray(x.transpose(0, 2, 1)).astype(bf16)  # [B, D, T]
    qkv_wT = np.ascontiguousarray(qkv_w.T)                   # [D, 3D]
    wo = np.ascontiguousarray(out_w.T).astype(bf16)          # [D, D]
    bo = np.broadcast_to(out_b.reshape(1, D), (128, D)).astype(bf16)
    bo = np.ascontiguousarray(bo)

    half = HD // 2
    freq = (1.0 / (10000.0 ** (np.arange(half, dtype=np.float64) / half)))
    ang = freq[:, None] * np.arange(t_total, dtype=np.float64)[None, :]
    cos_h = np.cos(ang)
    sin_h = np.sin(ang)
    cosT = np.concatenate([cos_h, cos_h], axis=0).astype(bf16)
    sinT = np.concatenate([sin_h, sin_h], axis=0).astype(bf16)

    P = np.zeros((HD, HD), dtype=f32)
    P[np.arange(half), np.arange(half) + half] = -1.0
    P[np.arange(half) + half, np.arange(half)] = 1.0
    pt = np.ascontiguousarray(P.T).astype(bf16)

    mask = np.where(np.arange(HD)[:, None] > np.arange(HD)[None, :],
                    f32(0.0), f32(1.0)).astype(bf16)
    onec = np.ones((HD, 1), dtype=bf16)
    oner = np.ones((1, HD), dtype=bf16)

    in_maps = []
    for c in range(NCORES):
        g0 = c * HPC * HD          # first feature col of this core's heads
        wq_c = np.ascontiguousarray(qkv_wT[:, g0:g0 + HPC * HD]).astype(bf16)
        wk_c = np.ascontiguousarray(
            qkv_wT[:, D + g0:D + g0 + HPC * HD]).astype(bf16)
        wv_c = np.ascontiguousarray(
            qkv_wT[:, 2 * D + g0:2 * D + g0 + HPC * HD]).astype(bf16)
        bq_c = qkv_b[g0:g0 + HPC * HD].reshape(HPC, HD)
        bk_c = qkv_b[D + g0:D + g0 + HPC * HD].reshape(HPC, HD)
        # [HD, 2*HPC]: columns q_h0, q_h1, k_h0, k_h1
        bqk_c = np.stack([bq_c[0], bq_c[1], bk_c[0], bk_c[1]], axis=1)
        bv_c = qkv_b[2 * D + g0:2 * D + g0 + HPC * HD].reshape(1, HPC * HD)
        bv_c = np.concatenate([bv_c, bv_c], axis=1)   # both psum col-halves
        in_maps.append({
            "xT": xT, "wq": wq_c, "wk": wk_c, "wv": wv_c,
            "bqk": np.ascontiguousarray(bqk_c.astype(f32)),
            "bv": np.ascontiguousarray(bv_c.astype(bf16)),
            "wo": wo, "bo": bo, "cosT": cosT, "sinT": sinT,
            "pt": pt, "maskT": mask,
            "onec": onec, "oner": oner,
        })
    return in_maps


def kernel(x, qkv_w, qkv_b, out_w, out_b):
    from concourse.bass_utils import run_bass_kernel_spmd

    if "nc" not in _CACHE:
        _CACHE["nc"] = _build_module()
    nc = _CACHE["nc"]

    in_maps = _host_inputs(x, qkv_w, qkv_b, out_w, out_b)
    res = run_bass_kernel_spmd(nc, in_maps, core_ids=list(range(NCORES)))
    rpb = (B * T) // NCORES // B     # 256 rows per core per batch
    out = np.empty((B, T, D), dtype=np.float32)
    for c in range(NCORES):
        yc = res.results[c]["y"]
        for b in range(B):
            out[b, c * rpb:(c + 1) * rpb, :] = yc[b * rpb:(b + 1) * rpb]
    return out


# revision 37
# speedup vs baseline: 1.0184x; 1.0184x over previous
"""Multi-head causal self-attention (B=2, T=2048, D=2048, 16 heads, RoPE)
on 8 Trainium2 NeuronCores.

Sharding strategy
-----------------
* Phase 1+2 (QKV projection, RoPE, attention): tensor-parallel over heads —
  each core owns 2 of the 16 heads for both batch elements. Each core reads
  the full (host-transposed, bf16) x and only its slice of qkv_w, and
  computes q/k in a transposed [head_dim, t] layout so RoPE and the score
  matmuls need no on-device transposes.
* Attention is computed as sT[tk, tq] = k-tile^T-free @ q (so softmax sums
  are ones-matmuls on the PE and the attention output lands directly in
  oT[dv, t] layout), exp on the scalar engine, probabilities in bf16.
  All matmul operands are bf16 (weights, activations, probabilities);
  PSUM accumulation stays fp32, softmax denominators stay fp32.
* Phase 3: four on-device AllToAlls — one per (local head, batch) — swap
  the head-sharded oT into a sequence-sharded full-head oT (0.5 MiB/core
  each in bf16). Each fires as soon as its (head, batch) attention
  completes, so only the very last one is tail-exposed.
* Phase 4: output projection is data-parallel over rows — core c owns 256
  rows of each batch. It runs per batch: batch-0's projection overlaps
  batch-1's QKV/attention compute. out_w is resident in SBUF in bf16 and
  loaded once. Output bias is folded into the even-parity parking add on
  the vector engine. Host concatenates the 8 row-slices per batch.
"""

import numpy as np

B = 2
T = 2048
D = 2048
H = 16            # global heads
HD = 128          # head dim
NCORES = 8
HPC = H // NCORES  # heads per core
NKT = D // 128     # contraction tiles over the embedding dim
TCH = 512          # t-chunk (phase 1) / tq-chunk (phase 2) width
SCALE = 1.0 / np.sqrt(HD)

_CACHE = {}


def _build_module(t_total=T):
    import concourse.bacc as bacc
    import concourse.mybir as mybir
    import concourse.tile as tile

    F32 = mybir.dt.float32
    F32R = mybir.dt.float32r
    BF16 = mybir.dt.bfloat16
    ADD = mybir.AluOpType.add
    MULT = mybir.AluOpType.mult
    AF = mybir.ActivationFunctionType

    t_ch = t_total // TCH          # chunks per batch element (4)
    rows = B * t_total             # 4096
    rpc = rows // NCORES           # rows per core (512)
    rpb = rpc // B                 # rows per core per batch (256)
    n_rt = rpb // 128              # row tiles per core per batch (2)
    w = HPC * HD                   # per-core q/k/v feature width (256)
    n_fc = D // TCH                # output-projection column chunks (4)

    nc = bacc.Bacc("TRN2", target_bir_lowering=False, debug=False,
                   num_devices=NCORES)

    # ---- I/O ----
    xT = nc.dram_tensor("xT", [B, D, t_total], BF16, kind="ExternalInput")
    wq = nc.dram_tensor("wq", [D, w], BF16, kind="ExternalInput")
    wk = nc.dram_tensor("wk", [D, w], BF16, kind="ExternalInput")
    wv = nc.dram_tensor("wv", [D, w], BF16, kind="ExternalInput")
    bqk = nc.dram_tensor("bqk", [HD, 2 * HPC], F32, kind="ExternalInput")
    bv = nc.dram_tensor("bv", [1, 2 * w], BF16, kind="ExternalInput")
    wo = nc.dram_tensor("wo", [D, D], BF16, kind="ExternalInput")
    bo = nc.dram_tensor("bo", [128, D], BF16, kind="ExternalInput")
    cosT = nc.dram_tensor("cosT", [HD, t_total], BF16, kind="ExternalInput")
    sinT = nc.dram_tensor("sinT", [HD, t_total], BF16, kind="ExternalInput")
    pt = nc.dram_tensor("pt", [HD, HD], BF16, kind="ExternalInput")
    maskT = nc.dram_tensor("maskT", [HD, HD], BF16, kind="ExternalInput")
    onec = nc.dram_tensor("onec", [HD, 1], BF16, kind="ExternalInput")
    oner = nc.dram_tensor("oner", [1, HD], BF16, kind="ExternalInput")
    y = nc.dram_tensor("y", [rpc, D], F32, kind="ExternalOutput")

    with tile.TileContext(nc) as tc:
        frees = []

        def single(shape, dtype, name, flist=frees):
            t, free = tc.tile(shape, dtype, name=name)
            flist.append(free)
            return t

        # ---- constants + resident weights in SBUF ----
        cos_sb = single([HD, t_total], BF16, "cos_sb")
        sin_sb = single([HD, t_total], BF16, "sin_sb")
        pt_sb = single([HD, HD], BF16, "pt_sb")
        mask_sb = single([HD, HD], BF16, "mask_sb")
        onec_sb = single([HD, 1], BF16, "onec_sb")
        oner_sb = single([1, HD], BF16, "oner_sb")
        bqk_sb = single([HD, 2 * HPC], F32, "bqk_sb")
        bv_sb = single([1, 2 * w], BF16, "bv_sb")
        bo_sb = single([128, D], BF16, "bo_sb")
        # out_w resident for the whole kernel: [128, kt*2048 + fc*512] bf16
        wo_sb = single([128, NKT * D], BF16, "wo_sb")

        # All constant loads are deferred to after the first x-chunk DMAs
        # (below) so the first matmuls are not starved behind weight DMA.

        # ---- DRAM bounce buffers for the per-(head, batch) AllToAlls ----
        with tc.tile_pool(name="dram", bufs=1, space="DRAM") as dram:
            bounce_in = [[dram.tile([NCORES * HD, rpb], BF16,
                                    name=f"bounce_in{h}_{b}")
                          for b in range(B)] for h in range(HPC)]
            bounce_out = [[dram.tile([NCORES * HD, rpb], BF16,
                                     name=f"bounce_out{h}_{b}")
                           for b in range(B)] for h in range(HPC)]
            # the very last (head, batch) AllToAll is tail-exposed: split it
            # into two row-half collectives so the odd projection pass can
            # start as soon as the first half lands.
            bounce_in_sp = [dram.tile([NCORES * HD, 128], BF16,
                                      name=f"bounce_in_sp{rt}")
                            for rt in range(n_rt)]
            bounce_out_sp = [dram.tile([NCORES * HD, 128], BF16,
                                       name=f"bounce_out_sp{rt}")
                             for rt in range(n_rt)]

            # PSUM pools stay open for the whole kernel: 8 banks total
            # (qk shares slots with v; st shares with rot/bcast/out-proj).
            with tc.tile_pool(name="qk_ps", bufs=2, space="PSUM") as qk_ps, \
                 tc.tile_pool(name="rot_ps", bufs=1, space="PSUM") as rot_ps, \
                 tc.tile_pool(name="st_ps", bufs=3, space="PSUM") as st_ps, \
                 tc.tile_pool(name="ot_ps", bufs=2, space="PSUM") as ot_ps:
                v_ps = qk_ps

                with tc.tile_pool(name="xt", bufs=17) as xt_pool, \
                     tc.tile_pool(name="tmp", bufs=6) as tmp_pool, \
                     tc.tile_pool(name="et", bufs=6) as et_pool, \
                     tc.tile_pool(name="nrm", bufs=3) as nrm_pool, \
                     tc.tile_pool(name="ets", bufs=2) as ets_pool, \
                     tc.tile_pool(name="oto", bufs=5) as oto_pool, \
                     tc.tile_pool(name="oc", bufs=NKT + NKT // 2) as oc_pool, \
                     tc.tile_pool(name="osp", bufs=8) as osp_pool, \
                     tc.tile_pool(name="os", bufs=3) as os_pool:

                    wfrees = []
                    wq_sb = single([128, NKT * w], BF16, "wq_sb", wfrees)
                    wk_sb = single([128, NKT * w], BF16, "wk_sb", wfrees)
                    wv_sb = single([128, NKT * w], BF16, "wv_sb", wfrees)

                    oc = {}      # (b, kt) -> [128, rpb] bf16 oT tiles
                    osp = {}     # (b, fc, rt) -> parked even partials

                    def emit_phase1_chunk(b, j):
                        """QKV projection + RoPE for chunk j of batch b into
                        q_st/k_st/v_st (freed per batch by the caller)."""
                        if True:
                            tr = slice(j * TCH, (j + 1) * TCH)
                            xt = []
                            for kt in range(NKT):
                                xtile = xt_pool.tile([128, TCH], BF16,
                                                     name=f"xt{b}{j}_{kt}",
                                                     tag="xt")
                                nc.sync.dma_start(
                                    xtile[:],
                                    xT.ap()[b, kt * 128:(kt + 1) * 128, tr])
                                xt.append(xtile)
                                if b == 0 and j == 0:
                                    # only wq rides along with the first x
                                    # chunk — everything else is ordered
                                    # strictly by first use below.
                                    nc.sync.dma_start(
                                        wq_sb[:, kt * w:(kt + 1) * w],
                                        wq.ap()[kt * 128:(kt + 1) * 128, :])
                            if b == 0 and j == 0:
                                nc.sync.dma_start(bqk_sb[:], bqk.ap()[:, :])
                                nc.sync.dma_start(pt_sb[:], pt.ap()[:, :])
                                nc.sync.dma_start(cos_sb[:], cosT.ap()[:, :])
                                nc.sync.dma_start(sin_sb[:], sinT.ap()[:, :])
                                for kt in range(NKT):
                                    nc.sync.dma_start(
                                        wk_sb[:, kt * w:(kt + 1) * w],
                                        wk.ap()[kt * 128:(kt + 1) * 128, :])
                                nc.sync.dma_start(bv_sb[:], bv.ap()[:, :])
                                for kt in range(NKT):
                                    nc.sync.dma_start(
                                        wv_sb[:, kt * w:(kt + 1) * w],
                                        wv.ap()[kt * 128:(kt + 1) * 128, :])
                                nc.sync.dma_start(mask_sb[:], maskT.ap()[:, :])
                                nc.sync.dma_start(onec_sb[:], onec.ap()[:, :])
                                nc.sync.dma_start(oner_sb[:], oner.ap()[:, :])
                                nc.sync.dma_start(bo_sb[:], bo.ap()[:, :])
                            if b == 1:
                                # out_w (8 MiB bf16): load once, spread over
                                # batch-1's chunks — needed only by the
                                # projection passes which start later still.
                                k0 = j * (NKT // t_ch)
                                for kt in range(k0, k0 + NKT // t_ch):
                                    nc.sync.dma_start(
                                        wo_sb[:, kt * D:(kt + 1) * D],
                                        wo.ap()[kt * 128:(kt + 1) * 128, :])

                            for which, w_sb, store in (
                                ("q", wq_sb, q_st), ("k", wk_sb, k_st)):
                                for h in range(HPC):
                                    ps = qk_ps.tile([128, TCH], F32,
                                                    name=f"{which}ps{b}{j}{h}",
                                                    tag="qk")
                                    for kt in range(NKT):
                                        col = kt * w + h * HD
                                        nc.tensor.matmul(
                                            ps[:],
                                            w_sb[:, col:col + HD],
                                            xt[kt][:],
                                            start=(kt == 0),
                                            stop=(kt == NKT - 1))
                                    # bias (per-partition) + round to bf16
                                    bcol = h if which == "q" else HPC + h
                                    qtmp = tmp_pool.tile(
                                        [128, TCH], BF16,
                                        name=f"{which}t{b}{j}{h}", tag="tmp")
                                    nc.scalar.activation(
                                        qtmp[:], ps[:], AF.Identity,
                                        bias=bqk_sb[:, bcol:bcol + 1],
                                        scale=1.0)
                                    # rotate-half via permutation matmul
                                    rp = rot_ps.tile([128, TCH], F32,
                                                     name=f"rp{b}{j}{h}",
                                                     tag="rot")
                                    nc.tensor.matmul(rp[:], pt_sb[:], qtmp[:],
                                                     start=True, stop=True)
                                    t1 = tmp_pool.tile([128, TCH], BF16,
                                                       name=f"t1_{b}{j}{h}",
                                                       tag="tmp")
                                    nc.vector.tensor_tensor(
                                        t1[:], qtmp[:], cos_sb[:, tr], MULT)
                                    t2 = tmp_pool.tile([128, TCH], BF16,
                                                       name=f"t2_{b}{j}{h}",
                                                       tag="tmp")
                                    nc.vector.tensor_tensor(
                                        t2[:], rp[:], sin_sb[:, tr], MULT)
                                    nc.vector.tensor_tensor(
                                        store[h][:, tr], t1[:], t2[:], ADD)

                            # v in natural [t, dv] layout, two t-tiles/psum
                            for half in range(2):
                                pv = v_ps.tile([128, TCH], F32,
                                               name=f"vps{b}{j}{half}",
                                               tag="qk")
                                # bias for both column halves in one matmul
                                nc.tensor.matmul(
                                    pv[:], oner_sb[:], bv_sb[:],
                                    start=True, stop=False,
                                    skip_group_check=True)
                                for sub in range(2):
                                    ts = half * 2 + sub
                                    cs = sub * w
                                    for kt in range(NKT):
                                        nc.tensor.matmul(
                                            pv[:, cs:cs + w],
                                            xt[kt][:, ts * 128:(ts + 1) * 128],
                                            wv_sb[:, kt * w:(kt + 1) * w],
                                            start=False,
                                            stop=(kt == NKT - 1),
                                            skip_group_check=True)
                                for sub in range(2):
                                    ts = half * 2 + sub
                                    tt = j * 4 + ts   # global t-tile index
                                    for h in range(HPC):
                                        nc.scalar.copy(
                                            v_st[h][:, tt * 128:
                                                    (tt + 1) * 128],
                                            pv[:, sub * w + h * HD:
                                               sub * w + (h + 1) * HD])

                    def emit_attn_chunk(b, h, c, state):
                        """Chunk c of causal attention for (batch b, local
                        head h) — needs q of chunk c and k/v of chunks
                        0..c only, so it can interleave right behind the
                        phase-1 chunk that produced them. Each chunk's
                        denominator / normalize / scatter chain is deferred
                        until after the next chunk's first score matmuls so
                        the PE is not stalled on the DVE accumulate at
                        chunk boundaries. The last chunk scatters and fires
                        this (h, b) AllToAll."""
                        if True:
                            ot = ot_ps.tile([128, TCH], F32,
                                            name=f"ot{b}{h}{c}", tag="ot")
                            ets = ets_pool.tile([128, TCH], BF16,
                                                name=f"ets{b}{h}{c}",
                                                tag="ets")
                            kmax = 4 * c + 3
                            ets_of = {}

                            def emit_st(k, c=c, ets=ets, ets_of=ets_of):
                                """Score matmul + exp + mask + denominator
                                accumulate for k-block k."""
                                off = max(0, (k - 4 * c) * 128)
                                ksl = slice(k * 128, (k + 1) * 128)
                                st = st_ps.tile([128, TCH], F32,
                                                name=f"st{b}{h}{c}{k}",
                                                tag="st")
                                q0 = c * TCH
                                nc.tensor.matmul(
                                    st[:, off:TCH],
                                    k_st[h][:, ksl],
                                    q_st[h][:, q0 + off:q0 + TCH],
                                    start=True, stop=True,
                                    skip_group_check=True)
                                et = et_pool.tile([128, TCH], BF16,
                                                  name=f"et{b}{h}{c}{k}",
                                                  tag="et")
                                nc.scalar.activation(
                                    et[:, off:TCH], st[:, off:TCH],
                                    AF.Exp, bias=0.0, scale=float(SCALE))
                                if k >= 4 * c:
                                    # zero the not-yet-causal triangle
                                    nc.vector.tensor_tensor(
                                        et[:, off:off + 128],
                                        et[:, off:off + 128],
                                        mask_sb[:], MULT)
                                # denominator partial sums on the DVE
                                if k == 0:
                                    nc.vector.tensor_copy(ets[:], et[:])
                                else:
                                    nc.vector.tensor_tensor(
                                        ets[:, off:TCH],
                                        ets[:, off:TCH],
                                        et[:, off:TCH], ADD)
                                ets_of[k] = (et, off)

                            # software-pipelined: the score matmul for k+1
                            # is emitted before the AV matmul for k, so the
                            # in-order PE has ready work while exp(k) is
                            # still on the scalar engine.
                            emit_st(0)
                            if state["fin"] is not None:
                                state["fin"]()
                                state["fin"] = None
                            for k in range(kmax + 1):
                                if k < kmax:
                                    emit_st(k + 1)
                                et, off = ets_of.pop(k)
                                ksl = slice(k * 128, (k + 1) * 128)
                                nc.tensor.matmul(
                                    ot[:, off:TCH],
                                    v_st[h][:, ksl],
                                    et[:, off:TCH],
                                    start=(k == 0), stop=(k == kmax),
                                    skip_group_check=True)

                            def finalize(c=c, ot=ot, ets=ets):
                                # den shares the rot/bc bank — their
                                # lifetimes never overlap
                                den = rot_ps.tile([1, TCH], F32,
                                                  name=f"den{b}{h}{c}",
                                                  tag="rot")
                                nc.tensor.matmul(
                                    den[0:1, :], onec_sb[:], ets[:],
                                    start=True, stop=True,
                                    skip_group_check=True)
                                # normalize by the softmax denominator
                                rc = nrm_pool.tile([1, TCH], F32,
                                                   name=f"rc{b}{h}{c}",
                                                   tag="rc")
                                rscr = nrm_pool.tile([1, TCH], F32,
                                                     name=f"rscr{b}{h}{c}",
                                                     tag="rc")
                                nc.vector.reciprocal_approx_accurate(
                                    rc[:], den[0:1, :], rscr[:])
                                rcr = nrm_pool.tile([1, TCH], BF16,
                                                    name=f"rcr{b}{h}{c}",
                                                    tag="rcr")
                                nc.scalar.copy(rcr[:], rc[:])
                                bc = rot_ps.tile([128, TCH], F32,
                                                 name=f"bc{b}{h}{c}",
                                                 tag="rot")
                                nc.tensor.matmul(bc[:], oner_sb[:], rcr[:],
                                                 start=True, stop=True,
                                                 skip_group_check=True)
                                bcs = nrm_pool.tile([128, TCH], F32,
                                                    name=f"bcs{b}{h}{c}",
                                                    tag="bcs")
                                nc.scalar.copy(bcs[:], bc[:])
                                otn = oto_pool.tile([128, TCH], BF16,
                                                    name=f"otn{b}{h}{c}",
                                                    tag="otn")
                                nc.vector.tensor_tensor(
                                    otn[:], ot[:], bcs[:], MULT)
                                # chunk c covers cores 2c, 2c+1 of batch b
                                split = (b == B - 1 and h == HPC - 1)
                                for s2 in range(2):
                                    s = 2 * c + s2
                                    if split:
                                        for rt in range(n_rt):
                                            nc.sync.dma_start(
                                                bounce_in_sp[rt][
                                                    s * HD:(s + 1) * HD, :],
                                                otn[:,
                                                    s2 * rpb + rt * 128:
                                                    s2 * rpb +
                                                    (rt + 1) * 128])
                                    else:
                                        nc.sync.dma_start(
                                            bounce_in[h][b][
                                                s * HD:(s + 1) * HD, :],
                                            otn[:, s2 * rpb:(s2 + 1) * rpb])

                            if c < t_ch - 1:
                                state["fin"] = finalize
                                return
                            finalize()
                        if b == B - 1 and h == HPC - 1:
                            for rt in range(n_rt):
                                nc.gpsimd.collective_compute(
                                    "AllToAll",
                                    mybir.AluOpType.bypass,
                                    replica_groups=[list(range(NCORES))],
                                    ins=[bounce_in_sp[rt][:].opt()],
                                    outs=[bounce_out_sp[rt][:].opt()],
                                )
                        else:
                            nc.gpsimd.collective_compute(
                                "AllToAll",
                                mybir.AluOpType.bypass,
                                replica_groups=[list(range(NCORES))],
                                ins=[bounce_in[h][b][:].opt()],
                                outs=[bounce_out[h][b][:].opt()],
                            )

                    def emit_oc_loads(b, parities=(0, 1)):
                        """SBUF loads of the AllToAll'd oT tiles for batch b.
                        Global odim tile kt = 2*s + h (head-major)."""
                        for h in parities:
                            split = (b == B - 1 and h == HPC - 1)
                            if split:
                                # rt-major: all first-half tiles are queued
                                # before any second-half tile, so they only
                                # wait on the first half-AllToAll.
                                for rt in range(n_rt):
                                    for s in range(NCORES):
                                        kt = HPC * s + h
                                        t_ = oc_pool.tile(
                                            [128, 128], BF16,
                                            name=f"oc{b}_{kt}_{rt}",
                                            tag="oc")
                                        nc.sync.dma_start(
                                            t_[:],
                                            bounce_out_sp[rt][
                                                s * 128:(s + 1) * 128, :])
                                        oc[b, kt, rt] = t_
                                continue
                            for s in range(NCORES):
                                kt = HPC * s + h
                                t_ = oc_pool.tile([128, rpb], BF16,
                                                  name=f"oc{b}_{kt}",
                                                  tag="oc")
                                nc.sync.dma_start(
                                    t_[:],
                                    bounce_out[h][b][s * 128:(s + 1) * 128,
                                                     :])
                                oc[b, kt] = t_

                    def emit_proj_even_piece(b, fc, rt):
                        """One even-kt partial-sum tile for batch b (gated on
                        AllToAll (h=0, b)); bias folded into the parking add
                        so the psum group is pure matmul."""
                        po = st_ps.tile([128, TCH], F32,
                                        name=f"po{b}{fc}{rt}",
                                        tag="st")
                        for i in range(NCORES):
                            kt = HPC * i
                            nc.tensor.matmul(
                                po[:],
                                oc[b, kt][:, rt * 128:(rt + 1) * 128],
                                wo_sb[:, kt * D + fc * TCH:
                                      kt * D + (fc + 1) * TCH],
                                start=(i == 0),
                                stop=(i == NCORES - 1),
                                skip_group_check=True)
                        p_ = osp_pool.tile([128, TCH], BF16,
                                           name=f"osp{b}{fc}{rt}",
                                           tag="osp")
                        nc.vector.tensor_tensor(
                            p_[:], po[:],
                            bo_sb[:, fc * TCH:(fc + 1) * TCH], ADD)
                        osp[b, fc, rt] = p_

                    def emit_proj_even(b):
                        for fc in range(n_fc):
                            for rt in range(n_rt):
                                emit_proj_even_piece(b, fc, rt)

                    def emit_proj_odd_piece(b, fc, rt):
                        """One odd-kt partial-sum tile for batch b (gated on
                        AllToAll (h=1, b)), combined with the parked evens
                        on the DVE and stored to y."""
                        po = st_ps.tile([128, TCH], F32,
                                        name=f"po2_{b}{fc}{rt}",
                                        tag="st")
                        for i in range(NCORES):
                            kt = HPC * i + 1
                            if (b, kt, rt) in oc:
                                sta = oc[b, kt, rt][:, :]
                            else:
                                sta = oc[b, kt][:, rt * 128:(rt + 1) * 128]
                            nc.tensor.matmul(
                                po[:],
                                sta,
                                wo_sb[:, kt * D + fc * TCH:
                                      kt * D + (fc + 1) * TCH],
                                start=(i == 0),
                                stop=(i == NCORES - 1),
                                skip_group_check=True)
                        os_t = os_pool.tile([128, TCH], F32,
                                            name=f"os{b}{fc}{rt}",
                                            tag="osp")
                        nc.vector.tensor_tensor(
                            os_t[:], po[:], osp[b, fc, rt][:], ADD)
                        nc.sync.dma_start(
                            y.ap()[b * rpb + rt * 128:
                                   b * rpb + (rt + 1) * 128,
                                   fc * TCH:(fc + 1) * TCH],
                            os_t[:])

                    def emit_proj_odd(b):
                        """rt-outer so the first row-half of the split final
                        AllToAll unblocks work while the second lands."""
                        for rt in range(n_rt):
                            for fc in range(n_fc):
                                emit_proj_odd_piece(b, fc, rt)

                    # ================= schedule =================
                    # The last phase-1 chunk is deferred until after the
                    # attention chunks c=0..2 (which only need q/k/v of
                    # chunks 0..2), so its x DMA prefetches during the
                    # attention stream instead of stalling the PE.
                    # Batch-0's output projection runs inside batch-1's
                    # stream, long after its AllToAlls fired (robust to
                    # collective-latency variance).
                    for b in range(B):
                        bfrees = []
                        q_st = [single([128, t_total], BF16, f"q_st{b}{h}",
                                       bfrees) for h in range(HPC)]
                        k_st = [single([128, t_total], BF16, f"k_st{b}{h}",
                                       bfrees) for h in range(HPC)]
                        v_st = [single([128, t_total], BF16, f"v_st{b}{h}",
                                       bfrees) for h in range(HPC)]
                        states = [{"fin": None} for _ in range(HPC)]

                        for j in range(t_ch - 1):
                            emit_phase1_chunk(b, j)
                            if b == 1 and j == 0:
                                # batch-0 oT tiles are in flight (AllToAlls
                                # fired at the end of batch 0) — bring them
                                # into SBUF behind batch-1's first x loads.
                                emit_oc_loads(0)
                        for h in range(HPC):
                            for c in range(t_ch - 1):
                                emit_attn_chunk(b, h, c, states[h])
                        if b == 1:
                            emit_proj_even(0)
                            emit_proj_odd(0)
                        emit_phase1_chunk(b, t_ch - 1)
                        for h in range(HPC):
                            emit_attn_chunk(b, h, t_ch - 1, states[h])
                        for f in reversed(bfrees):
                            f()

                    # the even pass overlaps the split final AllToAll pair;
                    # the odd pass starts as soon as the first half lands.
                    emit_oc_loads(1, (0,))
                    emit_proj_even(1)
                    emit_oc_loads(1, (1,))
                    emit_proj_odd(1)

                    for f in reversed(wfrees):
                        f()

        for f in reversed(frees):
            f()

    nc.compile()
    return nc


def _host_inputs(x, qkv_w, qkv_b, out_w, out_b, t_total=T):
    """Build the per-core input maps (all host-side layout shuffling)."""
    import ml_dtypes

    f32 = np.float32
    bf16 = ml_dtypes.bfloat16

    x = np.asarray(x, dtype=f32)
    qkv_w = np.asarray(qkv_w, dtype=f32)
    qkv_b = np.asarray(qkv_b, dtype=f32)
    out_w = np.asarray(out_w, dtype=f32)
    out_b = np.asarray(out_b, dtype=f32)

    xT = np.ascontiguousarray(x.transpose(0, 2, 1)).astype(bf16)  # [B, D, T]
    qkv_wT = np.ascontiguousarray(qkv_w.T)                   # [D, 3D]
    wo = np.ascontiguousarray(out_w.T).astype(bf16)          # [D, D]
    bo = np.broadcast_to(out_b.reshape(1, D), (128, D)).astype(bf16)
    bo = np.ascontiguousarray(bo)

    half = HD // 2
    freq = (1.0 / (10000.0 ** (np.arange(half, dtype=np.float64) / half)))
    ang = freq[:, None] * np.arange(t_total, dtype=np.float64)[None, :]
    cos_h = np.cos(ang)
    sin_h = np.sin(ang)
    cosT = np.concatenate([cos_h, cos_h], axis=0).astype(bf16)
    sinT = np.concatenate([sin_h, sin_h], axis=0).astype(bf16)

    P = np.zeros((HD, HD), dtype=f32)
    P[np.arange(half), np.arange(half) + half] = -1.0
    P[np.arange(half) + half, np.arange(half)] = 1.0
    pt = np.ascontiguousarray(P.T).astype(bf16)

    mask = np.where(np.arange(HD)[:, None] > np.arange(HD)[None, :],
                    f32(0.0), f32(1.0)).astype(bf16)
    onec = np.ones((HD, 1), dtype=bf16)
    oner = np.ones((1, HD), dtype=bf16)

    in_maps = []
    for c in range(NCORES):
        g0 = c * HPC * HD          # first feature col of this core's heads
        wq_c = np.ascontiguousarray(qkv_wT[:, g0:g0 + HPC * HD]).astype(bf16)
        wk_c = np.ascontiguousarray(
            qkv_wT[:, D + g0:D + g0 + HPC * HD]).astype(bf16)
        wv_c = np.ascontiguousarray(
            qkv_wT[:, 2 * D + g0:2 * D + g0 + HPC * HD]).astype(bf16)
        bq_c = qkv_b[g0:g0 + HPC * HD].reshape(HPC, HD)
        bk_c = qkv_b[D + g0:D + g0 + HPC * HD].reshape(HPC, HD)
        # [HD, 2*HPC]: columns q_h0, q_h1, k_h0, k_h1
        bqk_c = np.stack([bq_c[0], bq_c[1], bk_c[0], bk_c[1]], axis=1)
        bv_c = qkv_b[2 * D + g0:2 * D + g0 + HPC * HD].reshape(1, HPC * HD)
        bv_c = np.concatenate([bv_c, bv_c], axis=1)   # both psum col-halves
        in_maps.append({
            "xT": xT, "wq": wq_c, "wk": wk_c, "wv": wv_c,
            "bqk": np.ascontiguousarray(bqk_c.astype(f32)),
            "bv": np.ascontiguousarray(bv_c.astype(bf16)),
            "wo": wo, "bo": bo, "cosT": cosT, "sinT": sinT,
            "pt": pt, "maskT": mask,
            "onec": onec, "oner": oner,
        })
    return in_maps


def kernel(x, qkv_w, qkv_b, out_w, out_b):
    from concourse.bass_utils import run_bass_kernel_spmd

    if "nc" not in _CACHE:
        _CACHE["nc"] = _build_module()
    nc = _CACHE["nc"]

    in_maps = _host_inputs(x, qkv_w, qkv_b, out_w, out_b)
    res = run_bass_kernel_spmd(nc, in_maps, core_ids=list(range(NCORES)))
    rpb = (B * T) // NCORES // B     # 256 rows per core per batch
    out = np.empty((B, T, D), dtype=np.float32)
    for c in range(NCORES):
        yc = res.results[c]["y"]
        for b in range(B):
            out[b, c * rpb:(c + 1) * rpb, :] = yc[b * rpb:(b + 1) * rpb]
    return out


# revision 42
# speedup vs baseline: 1.1387x; 1.1181x over previous
"""Multi-head causal self-attention (B=2, T=2048, D=2048, 16 heads, RoPE)
on 8 Trainium2 NeuronCores.

Sharding strategy
-----------------
* Phase 1+2 (QKV projection, RoPE, attention): tensor-parallel over heads —
  each core owns 2 of the 16 heads for both batch elements. Each core reads
  the full (host-transposed, bf16) x and only its slice of qkv_w, and
  computes q/k in a transposed [head_dim, t] layout so RoPE and the score
  matmuls need no on-device transposes.
* Attention is computed as sT[tk, tq] = k-tile^T-free @ q (so softmax sums
  are ones-matmuls on the PE and the attention output lands directly in
  oT[dv, t] layout), exp on the scalar engine, probabilities in bf16.
  All matmul operands are bf16 (weights, activations, probabilities);
  PSUM accumulation stays fp32, softmax denominators stay fp32.
* Phase 3: four on-device AllToAlls — one per (local head, batch) — swap
  the head-sharded oT into a sequence-sharded full-head oT (0.5 MiB/core
  each in bf16). Each fires as soon as its (head, batch) attention
  completes, so only the very last one is tail-exposed.
* Phase 4: output projection is data-parallel over rows — core c owns 256
  rows of each batch. It runs per batch: batch-0's projection overlaps
  batch-1's QKV/attention compute. out_w is resident in SBUF in bf16 and
  loaded once. Output bias is folded into the even-parity parking add on
  the vector engine. Host concatenates the 8 row-slices per batch.
"""

import numpy as np

B = 2
T = 2048
D = 2048
H = 16            # global heads
HD = 128          # head dim
NCORES = 8
HPC = H // NCORES  # heads per core
NKT = D // 128     # contraction tiles over the embedding dim
TCH = 512          # t-chunk (phase 1) / tq-chunk (phase 2) width
SCALE = 1.0 / np.sqrt(HD)

_CACHE = {}


def _build_module(t_total=T):
    import concourse.bacc as bacc
    import concourse.mybir as mybir
    import concourse.tile as tile

    F32 = mybir.dt.float32
    F32R = mybir.dt.float32r
    BF16 = mybir.dt.bfloat16
    ADD = mybir.AluOpType.add
    MULT = mybir.AluOpType.mult
    AF = mybir.ActivationFunctionType

    t_ch = t_total // TCH          # chunks per batch element (4)
    rows = B * t_total             # 4096
    rpc = rows // NCORES           # rows per core (512)
    rpb = rpc // B                 # rows per core per batch (256)
    n_rt = rpb // 128              # row tiles per core per batch (2)
    w = HPC * HD                   # per-core q/k/v feature width (256)
    n_fc = D // TCH                # output-projection column chunks (4)

    nc = bacc.Bacc("TRN2", target_bir_lowering=False, debug=False,
                   num_devices=NCORES)

    # ---- I/O ----
    xT = nc.dram_tensor("xT", [B, D, t_total], BF16, kind="ExternalInput")
    wq = nc.dram_tensor("wq", [D, w], BF16, kind="ExternalInput")
    wk = nc.dram_tensor("wk", [D, w], BF16, kind="ExternalInput")
    wv = nc.dram_tensor("wv", [D, w], BF16, kind="ExternalInput")
    bqk = nc.dram_tensor("bqk", [HD, 2 * HPC], F32, kind="ExternalInput")
    bv = nc.dram_tensor("bv", [1, 2 * w], BF16, kind="ExternalInput")
    wo = nc.dram_tensor("wo", [D, D], BF16, kind="ExternalInput")
    bo = nc.dram_tensor("bo", [128, D], BF16, kind="ExternalInput")
    cosT = nc.dram_tensor("cosT", [HD, t_total], BF16, kind="ExternalInput")
    sinT = nc.dram_tensor("sinT", [HD, t_total], BF16, kind="ExternalInput")
    pt = nc.dram_tensor("pt", [HD, HD], BF16, kind="ExternalInput")
    maskT = nc.dram_tensor("maskT", [HD, HD], BF16, kind="ExternalInput")
    onec = nc.dram_tensor("onec", [HD, 1], BF16, kind="ExternalInput")
    oner = nc.dram_tensor("oner", [1, HD], BF16, kind="ExternalInput")
    y = nc.dram_tensor("y", [rpc, D], F32, kind="ExternalOutput")

    with tile.TileContext(nc) as tc:
        frees = []

        def single(shape, dtype, name, flist=frees):
            t, free = tc.tile(shape, dtype, name=name)
            flist.append(free)
            return t

        # ---- constants + resident weights in SBUF ----
        cos_sb = single([HD, t_total], BF16, "cos_sb")
        sin_sb = single([HD, t_total], BF16, "sin_sb")
        pt_sb = single([HD, HD], BF16, "pt_sb")
        mask_sb = single([HD, HD], BF16, "mask_sb")
        onec_sb = single([HD, 1], BF16, "onec_sb")
        oner_sb = single([1, HD], BF16, "oner_sb")
        bqk_sb = single([HD, 2 * HPC], F32, "bqk_sb")
        bv_sb = single([1, 2 * w], BF16, "bv_sb")
        bo_sb = single([128, D], BF16, "bo_sb")
        # out_w resident for the whole kernel: [128, kt*2048 + fc*512] bf16
        wo_sb = single([128, NKT * D], BF16, "wo_sb")

        # All constant loads are deferred to after the first x-chunk DMAs
        # (below) so the first matmuls are not starved behind weight DMA.

        # ---- DRAM bounce buffers for the per-(head, batch) AllToAlls ----
        with tc.tile_pool(name="dram", bufs=1, space="DRAM") as dram:
            bounce_in = [[dram.tile([NCORES * HD, rpb], BF16,
                                    name=f"bounce_in{h}_{b}")
                          for b in range(B)] for h in range(HPC)]
            bounce_out = [[dram.tile([NCORES * HD, rpb], BF16,
                                     name=f"bounce_out{h}_{b}")
                           for b in range(B)] for h in range(HPC)]
            # the very last (head, batch) AllToAll is tail-exposed: split it
            # into two row-half collectives so the odd projection pass can
            # start as soon as the first half lands.
            bounce_in_sp = [dram.tile([NCORES * HD, 128], BF16,
                                      name=f"bounce_in_sp{rt}")
                            for rt in range(n_rt)]
            bounce_out_sp = [dram.tile([NCORES * HD, 128], BF16,
                                       name=f"bounce_out_sp{rt}")
                             for rt in range(n_rt)]

            # PSUM pools stay open for the whole kernel: 8 banks total
            # (qk shares slots with v; st shares with rot/bcast/out-proj).
            with tc.tile_pool(name="qk_ps", bufs=2, space="PSUM") as qk_ps, \
                 tc.tile_pool(name="rot_ps", bufs=1, space="PSUM") as rot_ps, \
                 tc.tile_pool(name="st_ps", bufs=3, space="PSUM") as st_ps, \
                 tc.tile_pool(name="ot_ps", bufs=2, space="PSUM") as ot_ps:
                v_ps = qk_ps

                with tc.tile_pool(name="xt", bufs=25) as xt_pool, \
                     tc.tile_pool(name="tmp", bufs=6) as tmp_pool, \
                     tc.tile_pool(name="et", bufs=5) as et_pool, \
                     tc.tile_pool(name="nrm", bufs=3) as nrm_pool, \
                     tc.tile_pool(name="ets", bufs=2) as ets_pool, \
                     tc.tile_pool(name="oto", bufs=4) as oto_pool, \
                     tc.tile_pool(name="oc", bufs=NKT + 4) as oc_pool, \
                     tc.tile_pool(name="osp", bufs=8) as osp_pool, \
                     tc.tile_pool(name="os", bufs=3) as os_pool:

                    wfrees = []
                    wq_sb = single([128, NKT * w], BF16, "wq_sb", wfrees)
                    wk_sb = single([128, NKT * w], BF16, "wk_sb", wfrees)
                    wv_sb = single([128, NKT * w], BF16, "wv_sb", wfrees)

                    oc = {}      # (b, kt) -> [128, rpb] bf16 oT tiles
                    osp = {}     # (b, fc, rt) -> parked even partials

                    def emit_phase1(b):
                        """QKV projection + RoPE for batch b into
                        q_st/k_st/v_st (freed per batch by the caller)."""
                        for j in range(t_ch):
                            tr = slice(j * TCH, (j + 1) * TCH)
                            xt = []
                            for kt in range(NKT):
                                xtile = xt_pool.tile([128, TCH], BF16,
                                                     name=f"xt{b}{j}_{kt}",
                                                     tag="xt")
                                nc.sync.dma_start(
                                    xtile[:],
                                    xT.ap()[b, kt * 128:(kt + 1) * 128, tr])
                                xt.append(xtile)
                                if b == 0 and j == 0:
                                    # only wq rides along with the first x
                                    # chunk — everything else is ordered
                                    # strictly by first use below.
                                    nc.sync.dma_start(
                                        wq_sb[:, kt * w:(kt + 1) * w],
                                        wq.ap()[kt * 128:(kt + 1) * 128, :])
                            if b == 0 and j == 0:
                                nc.sync.dma_start(bqk_sb[:], bqk.ap()[:, :])
                                nc.sync.dma_start(pt_sb[:], pt.ap()[:, :])
                                nc.sync.dma_start(cos_sb[:], cosT.ap()[:, :])
                                nc.sync.dma_start(sin_sb[:], sinT.ap()[:, :])
                                for kt in range(NKT):
                                    nc.sync.dma_start(
                                        wk_sb[:, kt * w:(kt + 1) * w],
                                        wk.ap()[kt * 128:(kt + 1) * 128, :])
                                nc.sync.dma_start(bv_sb[:], bv.ap()[:, :])
                                for kt in range(NKT):
                                    nc.sync.dma_start(
                                        wv_sb[:, kt * w:(kt + 1) * w],
                                        wv.ap()[kt * 128:(kt + 1) * 128, :])
                                nc.sync.dma_start(mask_sb[:], maskT.ap()[:, :])
                                nc.sync.dma_start(onec_sb[:], onec.ap()[:, :])
                                nc.sync.dma_start(oner_sb[:], oner.ap()[:, :])
                                nc.sync.dma_start(bo_sb[:], bo.ap()[:, :])
                            if b == 1:
                                # out_w (8 MiB bf16): load once, spread over
                                # batch-1's chunks — needed only by the
                                # projection passes which start later still.
                                k0 = j * (NKT // t_ch)
                                for kt in range(k0, k0 + NKT // t_ch):
                                    nc.sync.dma_start(
                                        wo_sb[:, kt * D:(kt + 1) * D],
                                        wo.ap()[kt * 128:(kt + 1) * 128, :])

                            for which, w_sb, store in (
                                ("q", wq_sb, q_st), ("k", wk_sb, k_st)):
                                for h in range(HPC):
                                    ps = qk_ps.tile([128, TCH], F32,
                                                    name=f"{which}ps{b}{j}{h}",
                                                    tag="qk")
                                    for kt in range(NKT):
                                        col = kt * w + h * HD
                                        nc.tensor.matmul(
                                            ps[:],
                                            w_sb[:, col:col + HD],
                                            xt[kt][:],
                                            start=(kt == 0),
                                            stop=(kt == NKT - 1))
                                    # bias (per-partition) + round to bf16
                                    bcol = h if which == "q" else HPC + h
                                    qtmp = tmp_pool.tile(
                                        [128, TCH], BF16,
                                        name=f"{which}t{b}{j}{h}", tag="tmp")
                                    nc.scalar.activation(
                                        qtmp[:], ps[:], AF.Identity,
                                        bias=bqk_sb[:, bcol:bcol + 1],
                                        scale=1.0)
                                    # rotate-half via permutation matmul
                                    rp = rot_ps.tile([128, TCH], F32,
                                                     name=f"rp{b}{j}{h}",
                                                     tag="rot")
                                    nc.tensor.matmul(rp[:], pt_sb[:], qtmp[:],
                                                     start=True, stop=True)
                                    t1 = tmp_pool.tile([128, TCH], BF16,
                                                       name=f"t1_{b}{j}{h}",
                                                       tag="tmp")
                                    nc.vector.tensor_tensor(
                                        t1[:], qtmp[:], cos_sb[:, tr], MULT)
                                    t2 = tmp_pool.tile([128, TCH], BF16,
                                                       name=f"t2_{b}{j}{h}",
                                                       tag="tmp")
                                    nc.vector.tensor_tensor(
                                        t2[:], rp[:], sin_sb[:, tr], MULT)
                                    nc.vector.tensor_tensor(
                                        store[h][:, tr], t1[:], t2[:], ADD)

                            # v in natural [t, dv] layout, two t-tiles/psum
                            for half in range(2):
                                pv = v_ps.tile([128, TCH], F32,
                                               name=f"vps{b}{j}{half}",
                                               tag="qk")
                                # bias for both column halves in one matmul
                                nc.tensor.matmul(
                                    pv[:], oner_sb[:], bv_sb[:],
                                    start=True, stop=False,
                                    skip_group_check=True)
                                for sub in range(2):
                                    ts = half * 2 + sub
                                    cs = sub * w
                                    for kt in range(NKT):
                                        nc.tensor.matmul(
                                            pv[:, cs:cs + w],
                                            xt[kt][:, ts * 128:(ts + 1) * 128],
                                            wv_sb[:, kt * w:(kt + 1) * w],
                                            start=False,
                                            stop=(kt == NKT - 1),
                                            skip_group_check=True)
                                for sub in range(2):
                                    ts = half * 2 + sub
                                    tt = j * 4 + ts   # global t-tile index
                                    for h in range(HPC):
                                        nc.scalar.copy(
                                            v_st[h][:, tt * 128:
                                                    (tt + 1) * 128],
                                            pv[:, sub * w + h * HD:
                                               sub * w + (h + 1) * HD])

                    def emit_attention(b, h, backfill=None):
                        """Causal attention for (batch b, local head h);
                        scatters normalized oT chunks into bounce_in[h][b]
                        and fires that (h, b) AllToAll. `backfill[c]` emits
                        extra ready-to-run PE work after chunk c to cover
                        the exp-chain stalls. Each chunk's denominator /
                        normalize / scatter chain is deferred until after
                        the next chunk's first score matmuls so the PE is
                        not stalled on the DVE accumulate at chunk
                        boundaries."""
                        finalize_prev = None
                        for c in range(t_ch):
                            if backfill is not None:
                                for fn in backfill[c]:
                                    fn()
                            ot = ot_ps.tile([128, TCH], F32,
                                            name=f"ot{b}{h}{c}", tag="ot")
                            ets = ets_pool.tile([128, TCH], BF16,
                                                name=f"ets{b}{h}{c}",
                                                tag="ets")
                            kmax = 4 * c + 3
                            ets_of = {}

                            def emit_st(k, c=c, ets=ets, ets_of=ets_of):
                                """Score matmul + exp + mask + denominator
                                accumulate for k-block k."""
                                off = max(0, (k - 4 * c) * 128)
                                ksl = slice(k * 128, (k + 1) * 128)
                                st = st_ps.tile([128, TCH], F32,
                                                name=f"st{b}{h}{c}{k}",
                                                tag="st")
                                q0 = c * TCH
                                nc.tensor.matmul(
                                    st[:, off:TCH],
                                    k_st[h][:, ksl],
                                    q_st[h][:, q0 + off:q0 + TCH],
                                    start=True, stop=True,
                                    skip_group_check=True)
                                et = et_pool.tile([128, TCH], BF16,
                                                  name=f"et{b}{h}{c}{k}",
                                                  tag="et")
                                nc.scalar.activation(
                                    et[:, off:TCH], st[:, off:TCH],
                                    AF.Exp, bias=0.0, scale=float(SCALE))
                                if k >= 4 * c:
                                    # zero the not-yet-causal triangle
                                    nc.vector.tensor_tensor(
                                        et[:, off:off + 128],
                                        et[:, off:off + 128],
                                        mask_sb[:], MULT)
                                # denominator partial sums on the DVE
                                if k == 0:
                                    nc.vector.tensor_copy(ets[:], et[:])
                                else:
                                    nc.vector.tensor_tensor(
                                        ets[:, off:TCH],
                                        ets[:, off:TCH],
                                        et[:, off:TCH], ADD)
                                ets_of[k] = (et, off)

                            # software-pipelined: the score matmul for k+1
                            # is emitted before the AV matmul for k, so the
                            # in-order PE has ready work while exp(k) is
                            # still on the scalar engine.
                            emit_st(0)
                            if finalize_prev is not None:
                                finalize_prev()
                                finalize_prev = None
                            for k in range(kmax + 1):
                                if k < kmax:
                                    emit_st(k + 1)
                                et, off = ets_of.pop(k)
                                ksl = slice(k * 128, (k + 1) * 128)
                                nc.tensor.matmul(
                                    ot[:, off:TCH],
                                    v_st[h][:, ksl],
                                    et[:, off:TCH],
                                    start=(k == 0), stop=(k == kmax),
                                    skip_group_check=True)

                            def finalize(c=c, ot=ot, ets=ets):
                                # den shares the rot/bc bank — their
                                # lifetimes never overlap
                                den = rot_ps.tile([1, TCH], F32,
                                                  name=f"den{b}{h}{c}",
                                                  tag="rot")
                                nc.tensor.matmul(
                                    den[0:1, :], onec_sb[:], ets[:],
                                    start=True, stop=True,
                                    skip_group_check=True)
                                # normalize by the softmax denominator
                                rc = nrm_pool.tile([1, TCH], F32,
                                                   name=f"rc{b}{h}{c}",
                                                   tag="rc")
                                rscr = nrm_pool.tile([1, TCH], F32,
                                                     name=f"rscr{b}{h}{c}",
                                                     tag="rc")
                                nc.vector.reciprocal_approx_accurate(
                                    rc[:], den[0:1, :], rscr[:])
                                rcr = nrm_pool.tile([1, TCH], BF16,
                                                    name=f"rcr{b}{h}{c}",
                                                    tag="rcr")
                                nc.scalar.copy(rcr[:], rc[:])
                                bc = rot_ps.tile([128, TCH], F32,
                                                 name=f"bc{b}{h}{c}",
                                                 tag="rot")
                                nc.tensor.matmul(bc[:], oner_sb[:], rcr[:],
                                                 start=True, stop=True,
                                                 skip_group_check=True)
                                bcs = nrm_pool.tile([128, TCH], F32,
                                                    name=f"bcs{b}{h}{c}",
                                                    tag="bcs")
                                nc.scalar.copy(bcs[:], bc[:])
                                otn = oto_pool.tile([128, TCH], BF16,
                                                    name=f"otn{b}{h}{c}",
                                                    tag="otn")
                                nc.vector.tensor_tensor(
                                    otn[:], ot[:], bcs[:], MULT)
                                # chunk c covers cores 2c, 2c+1 of batch b
                                split = (b == B - 1 and h == HPC - 1)
                                for s2 in range(2):
                                    s = 2 * c + s2
                                    if split:
                                        for rt in range(n_rt):
                                            nc.sync.dma_start(
                                                bounce_in_sp[rt][
                                                    s * HD:(s + 1) * HD, :],
                                                otn[:,
                                                    s2 * rpb + rt * 128:
                                                    s2 * rpb +
                                                    (rt + 1) * 128])
                                    else:
                                        nc.sync.dma_start(
                                            bounce_in[h][b][
                                                s * HD:(s + 1) * HD, :],
                                            otn[:, s2 * rpb:(s2 + 1) * rpb])

                            if c < t_ch - 1:
                                finalize_prev = finalize
                            else:
                                finalize()
                        if b == B - 1 and h == HPC - 1:
                            for rt in range(n_rt):
                                nc.gpsimd.collective_compute(
                                    "AllToAll",
                                    mybir.AluOpType.bypass,
                                    replica_groups=[list(range(NCORES))],
                                    ins=[bounce_in_sp[rt][:].opt()],
                                    outs=[bounce_out_sp[rt][:].opt()],
                                )
                        else:
                            nc.gpsimd.collective_compute(
                                "AllToAll",
                                mybir.AluOpType.bypass,
                                replica_groups=[list(range(NCORES))],
                                ins=[bounce_in[h][b][:].opt()],
                                outs=[bounce_out[h][b][:].opt()],
                            )

                    def emit_oc_loads(b, parities=(0, 1)):
                        """SBUF loads of the AllToAll'd oT tiles for batch b.
                        Global odim tile kt = 2*s + h (head-major)."""
                        for h in parities:
                            split = (b == B - 1 and h == HPC - 1)
                            if split:
                                # rt-major: all first-half tiles are queued
                                # before any second-half tile, so they only
                                # wait on the first half-AllToAll.
                                for rt in range(n_rt):
                                    for s in range(NCORES):
                                        kt = HPC * s + h
                                        t_ = oc_pool.tile(
                                            [128, 128], BF16,
                                            name=f"oc{b}_{kt}_{rt}",
                                            tag="oc")
                                        nc.sync.dma_start(
                                            t_[:],
                                            bounce_out_sp[rt][
                                                s * 128:(s + 1) * 128, :])
                                        oc[b, kt, rt] = t_
                                continue
                            for s in range(NCORES):
                                kt = HPC * s + h
                                t_ = oc_pool.tile([128, rpb], BF16,
                                                  name=f"oc{b}_{kt}",
                                                  tag="oc")
                                nc.sync.dma_start(
                                    t_[:],
                                    bounce_out[h][b][s * 128:(s + 1) * 128,
                                                     :])
                                oc[b, kt] = t_

                    def emit_proj_even_piece(b, fc, rt):
                        """One even-kt partial-sum tile for batch b (gated on
                        AllToAll (h=0, b)); bias folded into the parking add
                        so the psum group is pure matmul."""
                        po = st_ps.tile([128, TCH], F32,
                                        name=f"po{b}{fc}{rt}",
                                        tag="st")
                        for i in range(NCORES):
                            kt = HPC * i
                            nc.tensor.matmul(
                                po[:],
                                oc[b, kt][:, rt * 128:(rt + 1) * 128],
                                wo_sb[:, kt * D + fc * TCH:
                                      kt * D + (fc + 1) * TCH],
                                start=(i == 0),
                                stop=(i == NCORES - 1),
                                skip_group_check=True)
                        p_ = osp_pool.tile([128, TCH], BF16,
                                           name=f"osp{b}{fc}{rt}",
                                           tag="osp")
                        nc.vector.tensor_tensor(
                            p_[:], po[:],
                            bo_sb[:, fc * TCH:(fc + 1) * TCH], ADD)
                        osp[b, fc, rt] = p_

                    def emit_proj_even(b):
                        for fc in range(n_fc):
                            for rt in range(n_rt):
                                emit_proj_even_piece(b, fc, rt)

                    def emit_proj_odd(b):
                        """Odd-kt partial sums for batch b's rows (gated on
                        AllToAll (h=1, b)), combined with the parked evens
                        on the DVE and stored to y. rt-outer so the first
                        row-half of the split final AllToAll unblocks work
                        while the second is in flight."""
                        for rt in range(n_rt):
                            for fc in range(n_fc):
                                po = st_ps.tile([128, TCH], F32,
                                                name=f"po2_{b}{fc}{rt}",
                                                tag="st")
                                for i in range(NCORES):
                                    kt = HPC * i + 1
                                    if (b, kt, rt) in oc:
                                        sta = oc[b, kt, rt][:, :]
                                    else:
                                        sta = oc[b, kt][:, rt * 128:
                                                        (rt + 1) * 128]
                                    nc.tensor.matmul(
                                        po[:],
                                        sta,
                                        wo_sb[:, kt * D + fc * TCH:
                                              kt * D + (fc + 1) * TCH],
                                        start=(i == 0),
                                        stop=(i == NCORES - 1),
                                        skip_group_check=True)
                                os_t = os_pool.tile([128, TCH], F32,
                                                    name=f"os{b}{fc}{rt}",
                                                    tag="osp")
                                nc.vector.tensor_tensor(
                                    os_t[:], po[:], osp[b, fc, rt][:], ADD)
                                nc.sync.dma_start(
                                    y.ap()[b * rpb + rt * 128:
                                           b * rpb + (rt + 1) * 128,
                                           fc * TCH:(fc + 1) * TCH],
                                    os_t[:])

                    # ================= schedule =================
                    for b in range(B):
                        bfrees = []
                        q_st = [single([128, t_total], BF16, f"q_st{b}{h}",
                                       bfrees) for h in range(HPC)]
                        k_st = [single([128, t_total], BF16, f"k_st{b}{h}",
                                       bfrees) for h in range(HPC)]
                        v_st = [single([128, t_total], BF16, f"v_st{b}{h}",
                                       bfrees) for h in range(HPC)]

                        emit_phase1(b)
                        if b == 1:
                            # batch-0 oT tiles are in flight (AllToAlls fired
                            # during batch-0 attention) — bring them into
                            # SBUF behind batch-1's x loads.
                            emit_oc_loads(0)
                        for h in range(HPC):
                            if b == 1 and h == 1:
                                # prefetch batch-1's even oT tiles (their
                                # AllToAll fired after the previous head)
                                emit_oc_loads(1, (0,))
                            emit_attention(b, h)
                            if b == 1 and h == 0:
                                # batch-0 projection fills the PE while
                                # batch-1's last AllToAlls are in flight.
                                emit_proj_even(0)
                                emit_proj_odd(0)
                        for f in reversed(bfrees):
                            f()

                    # the even pass overlaps the split final AllToAll pair;
                    # the odd pass starts as soon as the first half lands.
                    emit_proj_even(1)
                    emit_oc_loads(1, (1,))
                    emit_proj_odd(1)

                    for f in reversed(wfrees):
                        f()

        for f in reversed(frees):
            f()

    nc.compile()
    return nc


def _host_inputs(x, qkv_w, qkv_b, out_w, out_b, t_total=T):
    """Build the per-core input maps (all host-side layout shuffling)."""
    import ml_dtypes

    f32 = np.float32
    bf16 = ml_dtypes.bfloat16

    x = np.asarray(x, dtype=f32)
    qkv_w = np.asarray(qkv_w, dtype=f32)
    qkv_b = np.asarray(qkv_b, dtype=f32)
    out_w = np.asarray(out_w, dtype=f32)
    out_b = np.asarray(out_b, dtype=f32)

    xT = np.ascontiguousarray(x.transpose(0, 2, 1)).astype(bf16)  # [B, D, T]
    qkv_wT = np.ascontiguousarray(qkv_w.T)                   # [D, 3D]
    wo = np.ascontiguousarray(out_w.T).astype(bf16)          # [D, D]
    bo = np.broadcast_to(out_b.reshape(1, D), (128, D)).astype(bf16)
    bo = np.ascontiguousarray(bo)

    half = HD // 2
    freq = (1.0 / (10000.0 ** (np.arange(half, dtype=np.float64) / half)))
    ang = freq[:, None] * np.arange(t_total, dtype=np.float64)[None, :]
    cos_h = np.cos(ang)
    sin_h = np.sin(ang)
    cosT = np.concatenate([cos_h, cos_h], axis=0).astype(bf16)
    sinT = np.concatenate([sin_h, sin_h], axis=0).astype(bf16)

    P = np.zeros((HD, HD), dtype=f32)
    P[np.arange(half), np.arange(half) + half] = -1.0
    P[np.arange(half) + half, np.arange(half)] = 1.0
    pt = np.ascontiguousarray(P.T).astype(bf16)

    mask = np.where(np.arange(HD)[:, None] > np.arange(HD)[None, :],
                    f32(0.0), f32(1.0)).astype(bf16)
    onec = np.ones((HD, 1), dtype=bf16)
    oner = np.ones((1, HD), dtype=bf16)

    in_maps = []
    for c in range(NCORES):
        g0 = c * HPC * HD          # first feature col of this core's heads
        wq_c = np.ascontiguousarray(qkv_wT[:, g0:g0 + HPC * HD]).astype(bf16)
        wk_c = np.ascontiguousarray(
            qkv_wT[:, D + g0:D + g0 + HPC * HD]).astype(bf16)
        wv_c = np.ascontiguousarray(
            qkv_wT[:, 2 * D + g0:2 * D + g0 + HPC * HD]).astype(bf16)
        bq_c = qkv_b[g0:g0 + HPC * HD].reshape(HPC, HD)
        bk_c = qkv_b[D + g0:D + g0 + HPC * HD].reshape(HPC, HD)
        # [HD, 2*HPC]: columns q_h0, q_h1, k_h0, k_h1
        bqk_c = np.stack([bq_c[0], bq_c[1], bk_c[0], bk_c[1]], axis=1)
        bv_c = qkv_b[2 * D + g0:2 * D + g0 + HPC * HD].reshape(1, HPC * HD)
        bv_c = np.concatenate([bv_c, bv_c], axis=1)   # both psum col-halves
        in_maps.append({
            "xT": xT, "wq": wq_c, "wk": wk_c, "wv": wv_c,
            "bqk": np.ascontiguousarray(bqk_c.astype(f32)),
            "bv": np.ascontiguousarray(bv_c.astype(bf16)),
            "wo": wo, "bo": bo, "cosT": cosT, "sinT": sinT,
            "pt": pt, "maskT": mask,
            "onec": onec, "oner": oner,
        })
    return in_maps


def kernel(x, qkv_w, qkv_b, out_w, out_b):
    from concourse.bass_utils import run_bass_kernel_spmd

    if "nc" not in _CACHE:
        _CACHE["nc"] = _build_module()
    nc = _CACHE["nc"]

    in_maps = _host_inputs(x, qkv_w, qkv_b, out_w, out_b)
    res = run_bass_kernel_spmd(nc, in_maps, core_ids=list(range(NCORES)))
    rpb = (B * T) // NCORES // B     # 256 rows per core per batch
    out = np.empty((B, T, D), dtype=np.float32)
    for c in range(NCORES):
        yc = res.results[c]["y"]
        for b in range(B):
            out[b, c * rpb:(c + 1) * rpb, :] = yc[b * rpb:(b + 1) * rpb]
    return out
